# revision 57
# baseline (speedup 1.0000x reference)
"""PointPillarsScatter Trainium2 kernel (fp16, bandwidth-optimized).

Reference op:
  canvas[b*NY*NX + y*NX + x] = voxel_features[p]        (scatter-set, 64 ch)
  out[:, :64]  = canvas -> [B, 64, NY, NX]
  out[:, 64:]  = transpose(map_fm, (0, 3, 2, 1))        (16 ch)

Strategy (8 NeuronCores, SPMD), core = batch*2 + y_half:
  The op is pure data movement, so the kernel is sized by HBM traffic.
  Everything runs in fp16 (abs rel err ~2^-11, far inside the 2e-2
  gate): the 34MB/core of fp32 output becomes 17MB, and all input
  tables are fp16 too.

  Scatter = one-hot matmul on the TensorEngine:
    out[128ch', 512cells] = featT[K, 128ch'].T @ S[K, 512]
  with S[s, n] = (pos[s] == n) built by DVE is_equal against a
  DMA-loaded iota table.  ch' packs the 64 channels of TWO 512-cell
  tiles (tile 2j -> psum 0:64, tile 2j+1 -> 64:128) so one matmul
  covers 1024 contiguous cells; K (points per column) is the
  per-group max point count over cores, baked into the program.
  Pairs go two-at-a-time into 2-bank psum tiles; copies (ACT/DVE,
  fp32->fp16) drain into a staging tile that one full-width 2MB DMA
  writes out per group, in psum layout [blk*64+ch, pair*512+pos] --
  the host de-interleaves during the gather step.  Feat loads always
  use all 128 partitions (partial-partition DMAs serialize onto one
  SDMA engine at ~27GB/s).

  map_fm is transposed to [CM, NYH, NX] fp16 on the host; the device
  moves it with DRAM->DRAM DMA chunks on the scalar queue, spread
  across the groups so it never starves the scatter pipeline's head.

Host side only computes index tables + shards/reformats inputs (per
the sharding hint); all arithmetic that produces output values runs
on device.  The host up-casts the returned fp16 shards to fp32.
"""

import sys

for _p in ("/opt/trn_rl_repo",):
    if _p not in sys.path:
        sys.path.insert(0, _p)

import numpy as np

# problem constants (hardcoded per contract)
B, NPTS, C, NY, NX, CM = 4, 48000, 64, 496, 432, 16
NYH = NY // 2            # 248 rows per core
NCORE = 8
NCELL = NYH * NX         # 107136 cells per core
TILE = 512               # cells per channel-block
NT = (NCELL + TILE - 1) // TILE          # 210 tiles (last has 128 cells)
NP = NT // 2                             # 105 pairs: tile 2j with 2j+1
LAST = NCELL - (NT - 1) * TILE           # 128 cells in the last tile
CAP = 128                # max points per column (K dim of the matmul)
# pair counts per group: big groups amortize DMA, small last group
# shortens the final drain
GSIZES = [15, 15, 15, 15, 15, 15, 15]
GBOUND = [0]
for _gs in GSIZES:
    GBOUND.append(GBOUND[-1] + _gs)
assert GBOUND[-1] == NP
NGRP = len(GSIZES)
SGMAX = max(GSIZES)
FB = 18                  # column budget of the per-group feat tile

_prog_cache = {}


def _build_program(ncols, chunks, kg):
    """Build the SPMD Bass program (identical for all 8 cores)."""
    from concourse import bacc, mybir, tile

    f16 = mybir.dt.float16
    f32 = mybir.dt.float32

    nc = bacc.Bacc(trn_type="TRN2", target_bir_lowering=False)

    feat_d = nc.dram_tensor("feat", [CAP, ncols * 128], f16,
                            kind="ExternalInput")
    iota_d = nc.dram_tensor("iota", [CAP, TILE], f16, kind="ExternalInput")
    post_d = nc.dram_tensor("post", [CAP, ncols], f32, kind="ExternalInput")
    MAPW = CM * NCELL // 128
    map_d = nc.dram_tensor("mapin", [128, MAPW], f16, kind="ExternalInput")
    # canvas in psum layout: row = blk*64+ch, col = pair*512+pos; the
    # host de-interleaves tile halves during the gather step
    out_d = nc.dram_tensor("out", [128, NP * TILE], f16,
                           kind="ExternalOutput")
    outm_d = nc.dram_tensor("outm", [128, MAPW], f16,
                            kind="ExternalOutput")

    colbase = np.concatenate([[0], np.cumsum(chunks)]).astype(np.int64)

    with tile.TileContext(nc) as tc:
        with (
            tc.tile_pool(name="const", bufs=1) as cpool,
            tc.tile_pool(name="fpool", bufs=3) as fpool,
            tc.tile_pool(name="spool", bufs=4) as spool,
            tc.tile_pool(name="stg", bufs=4) as stpool,
            tc.tile_pool(name="pscat", bufs=3, space="PSUM") as pspool,
        ):
            # DMA-loaded iota table + per-column slot positions
            iota_f = cpool.tile([CAP, TILE], f16)
            nc.sync.dma_start(out=iota_f[:], in_=iota_d[:])
            posT = cpool.tile([CAP, ncols], f32)
            nc.scalar.dma_start(out=posT[:], in_=post_d[:])

            # map passthrough: DRAM->DRAM chunks on the scalar queue so
            # the map bandwidth fills the gaps of the scatter pipeline
            # instead of starving its head.
            NMCH = 7
            mchunk = (MAPW + NMCH - 1) // NMCH

            def emit_map_chunk(g, eng):
                if g >= NMCH:
                    return
                m0 = g * mchunk
                m1 = min(m0 + mchunk, MAPW)
                eng.dma_start(out=outm_d[:, m0:m1],
                              in_=map_d[:, m0:m1])

            def emit_feat_load(g, eng=None):
                p0, p1 = GBOUND[g], GBOUND[g + 1]
                c0, c1 = int(colbase[p0]), int(colbase[p1])
                fb = fpool.tile([CAP, FB * 128], f16, tag="fb")
                # sync queue, emitted before older groups' out DMAs so
                # the in-order sync engine issues the prefetch first.
                # Full 128 partitions: partial-partition DMAs serialize
                # onto a single SDMA engine (~27GB/s).
                e = eng or nc.sync
                if g == 0:
                    # tiny head load so pair 0's matmul starts ~3us
                    # earlier than the full 0.5MB group load allows
                    ch = int(colbase[min(p0 + 2, p1)]) - c0
                    e.dma_start(out=fb[:, :ch * 128],
                                in_=feat_d[:, c0 * 128:(c0 + ch) * 128])
                    e.dma_start(out=fb[:, ch * 128:(c1 - c0) * 128],
                                in_=feat_d[:, (c0 + ch) * 128:c1 * 128])
                else:
                    e.dma_start(out=fb[:, :(c1 - c0) * 128],
                                in_=feat_d[:, c0 * 128:c1 * 128])
                return fb

            # feat0 on the scalar queue: loads in parallel with iota/posT
            fbs = {0: emit_feat_load(0, nc.scalar), 1: emit_feat_load(1)}
            for g in range(NGRP):
                p0 = GBOUND[g]
                p1 = GBOUND[g + 1]
                c0 = int(colbase[p0])
                K = int(kg[g])
                fb = fbs.pop(g)
                if g + 2 < NGRP:
                    fbs[g + 2] = emit_feat_load(g + 2)
                emit_map_chunk(g, nc.scalar)
                stg = stpool.tile([128, SGMAX * TILE], f16, tag="stg")
                # pairs are processed two-at-a-time into a 2-bank psum
                # tile; the copy drains both pairs at once and is emitted
                # AFTER the next pair's mask so DVE keeps the PE fed.
                last = g == NGRP - 1

                def emit_copy(cps, coff, cw, idx):
                    if idx % 4 == 0:
                        nc.vector.tensor_copy(
                            out=stg[:, coff:coff + cw], in_=cps[:, :cw])
                    else:
                        nc.scalar.copy(
                            out=stg[:, coff:coff + cw], in_=cps[:, :cw])

                pend = None        # (psum tile, stg offset, width)
                for pr in range(p0, p1):
                    half = (pr - p0) % 2
                    if half == 0:
                        npair = min(2, p1 - pr)
                        ps = pspool.tile([128, npair * TILE], f32)
                    nck = int(chunks[pr])
                    for k in range(nck):
                        col = int(colbase[pr]) + k
                        s_t = spool.tile([CAP, TILE], f16)
                        nc.vector.tensor_scalar(
                            out=s_t[:K], in0=iota_f[:K, :],
                            scalar1=posT[:K, col:col + 1],
                            scalar2=None,
                            op0=mybir.AluOpType.is_equal)
                        lhs = fb[:K, (col - c0) * 128:(col - c0 + 1) * 128]
                        nc.tensor.matmul(
                            out=ps[:, half * TILE:(half + 1) * TILE],
                            lhsT=lhs, rhs=s_t[:K],
                            start=(k == 0), stop=(k == nck - 1))
                    if pend is not None:
                        cps, coff, cw = pend
                        pend = None
                        emit_copy(cps, coff, cw, pr // 2)
                    if half == npair - 1:
                        pend = (ps, (pr - p0 - half) * TILE, npair * TILE)
                if pend is not None:
                    cps, coff, cw = pend
                    emit_copy(cps, coff, cw, 1)
                # out DMA in psum layout, split in halves across both
                # queues: the parallel drain frees the staging buffer
                # twice as fast when HBM contention delays a queue
                oeng = nc.sync if g % 2 == 0 else nc.scalar
                other = nc.scalar if g % 2 == 0 else nc.sync
                wa = (p1 - p0) * TILE
                wh = (wa // (2 * TILE)) * TILE
                oeng.dma_start(out=out_d[:, p0 * TILE:p0 * TILE + wh],
                               in_=stg[:, :wh])
                other.dma_start(
                    out=out_d[:, p0 * TILE + wh:p0 * TILE + wa],
                    in_=stg[:, wh:wa])

    nc.finalize()
    return nc


def _host_prep(voxel_features, coords, map_fm):
    """Shard points by core, build feat/pos tables (host index work only)."""
    vf = np.asarray(voxel_features).astype(np.float16)
    cd = np.asarray(coords)
    mf = np.asarray(map_fm)
    if mf.ndim == 5:
        mf = np.squeeze(mf, 3)

    b = cd[:, 0].astype(np.int64)
    y = cd[:, 2].astype(np.int64)
    x = cd[:, 3].astype(np.int64)
    valid = (b >= 0) & (b < B) & (y >= 0) & (y < NY) & (x >= 0) & (x < NX)
    b, y, x = b[valid], y[valid], x[valid]
    vfv = vf[valid]

    half = (y >= NYH).astype(np.int64)
    core = b * 2 + half
    lcell = (y - half * NYH) * NX + x
    t = lcell // TILE          # 512-cell tile id
    pos = lcell - t * TILE     # position within tile (= matmul column)
    pair = t // 2              # tile 2j pairs with tile 2j+1
    blk = t % 2                # channel block within the pair

    key = core * NP + pair
    order = np.argsort(key, kind="stable")
    ks = key[order]
    counts = np.bincount(ks, minlength=NCORE * NP)
    maxcnt = counts.reshape(NCORE, NP).max(axis=0)
    chunks = np.maximum((maxcnt + CAP - 1) // CAP, 1)
    kpair = np.maximum((maxcnt + chunks - 1) // chunks, 1)
    ncols = int(chunks.sum())
    colbase = np.concatenate([[0], np.cumsum(chunks)]).astype(np.int64)

    kg = np.zeros(NGRP, np.int64)
    for g in range(NGRP):
        p0, p1 = GBOUND[g], GBOUND[g + 1]
        kg[g] = int(kpair[p0:p1].max())
        need = int(chunks[p0:p1].sum())
        if need > FB:
            raise ValueError("pair group needs %d cols > FB=%d" % (need, FB))

    starts = np.concatenate([[0], np.cumsum(counts)]).astype(np.int64)
    rank = np.arange(len(ks), dtype=np.int64) - starts[ks]

    co = core[order]
    po = pair[order]
    bo = blk[order]
    kp = kpair[po]
    colo = colbase[po] + rank // kp
    slot = rank % kp

    feat = np.zeros((NCORE, CAP, ncols * 128), np.float16)
    ccol = (colo * 128 + bo * C)[:, None] + np.arange(C)[None, :]
    feat[co[:, None], slot[:, None], ccol] = vfv[order]

    iota = np.broadcast_to(
        np.arange(TILE, dtype=np.float16)[None, :], (CAP, TILE)).copy()
    post = np.full((NCORE, CAP, ncols), -1.0, np.float32)
    post[co, slot, colo] = pos[order].astype(np.float32)

    maps = []
    for core_id in range(NCORE):
        bb, hh = core_id // 2, core_id % 2
        m = mf[bb, :, hh * NYH:(hh + 1) * NYH, :]      # [NX, NYH, CM]
        maps.append(np.ascontiguousarray(
            np.transpose(m, (2, 1, 0)).astype(np.float16))
            .reshape(128, CM * NCELL // 128))
    return feat, iota, post, maps, ncols, chunks, kg


def kernel(voxel_features, coords, batch_size=None, map_fm=None,
           trace=False, _return_results=False):
    from concourse.bass_utils import run_bass_kernel_spmd

    feat, iota, post, maps, ncols, chunks, kg = _host_prep(
        voxel_features, coords, map_fm)

    ckey = (ncols, tuple(int(c) for c in chunks), tuple(int(k) for k in kg))
    if ckey not in _prog_cache:
        _prog_cache.clear()
        _prog_cache[ckey] = _build_program(ncols, chunks, kg)
    nc = _prog_cache[ckey]

    in_maps = [
        {"feat": feat[i], "iota": iota, "post": post[i], "mapin": maps[i]}
        for i in range(NCORE)
    ]
    res = run_bass_kernel_spmd(nc, in_maps, list(range(NCORE)), trace=trace)

    out = np.empty((B, C + CM, NY, NX), np.float32)
    for core_id in range(NCORE):
        bb, hh = core_id // 2, core_id % 2
        blk = np.empty((C + CM, NCELL), np.float32)
        # de-interleave tile halves: psum row blk*64+ch, col pair*512+pos
        can = res.results[core_id]["out"].reshape(2, C, NP, TILE)
        blk[:C] = (np.transpose(can, (1, 2, 0, 3))
                   .reshape(C, NP * 2 * TILE)[:, :NCELL])
        blk[C:] = res.results[core_id]["outm"].reshape(CM, NCELL)
        out[bb, :, hh * NYH:(hh + 1) * NYH, :] = blk.reshape(C + CM, NYH, NX)
    if _return_results:
        return out, res
    return out
